# revision 23
# baseline (speedup 1.0000x reference)
"""MemNet (scatter_memory) Trainium2 kernel, v2.

Model (per batch row b):
  memory   = emb[context_x[b]]                    # [L, D] gather
  v_aspect = masked-mean(emb[target_x[b]])        # [D]
  v_loc    = 1 - |pos - target_loc[b]| / context_len[b]
  3 hops of: scores = tanh((memory*v_loc) @ w_mem + vec@w_vec + b)
             alpha  = masked softmax;  vec = alpha @ (memory*v_loc) + vec@lin_w+lin_b
  logits   = vec @ out_w + out_b

Sharding: data-parallel over batch, 32 rows per core on 8 cores; the fp16
embedding table is index-compacted per core and fetched by indirect DMA
gather.

Key structure (vs v1):
- The content score emb@w_mem is a pure weight transform; it is appended
  as column 300 of the gathered rows (inside the 768B alignment padding),
  so scores arrive with the gather for free.
- Attention contraction runs transposed on the PE: per (chunk, d-slice),
  stationary = mem[128, DK], moving = the chunk's alpha column [128, 1],
  accumulating over each batch row's 4 chunks directly into vec^T layout
  [DK, b] in PSUM. Cost ~ 1 cycle per matmul (output free size 1).
- v_loc/cmask are host-side index-derived tensors; hop 1 (whose alpha
  depends only on v_aspect) is pipelined per gather group so its attention
  runs under the gather phase; the softmax denominator accumulates per
  group in an open PSUM matmul group.
"""

import numpy as np

import concourse.bass as bass
import concourse.bacc as bacc
import concourse.mybir as mybir
import concourse.tile as tile
from concourse import bass_utils

N_CORES = 8
B, L, T, V, D, C = 256, 512, 5, 50000, 300, 3
N_HOPS = 3
BP = B // N_CORES          # 32 batch rows per core
P = 128                    # partitions
NCH = (BP * L) // P        # 128 chunk columns; chunk c: b=c//4, l=(c%4)*128+p
CPB = L // P               # 4 chunks per batch row
NGRP = 16                  # gather groups (<=1024 idxs per dma_gather)
GW = NCH // NGRP           # chunk columns per gather group (8)
DK = [128, 128, 44]        # D split across PSUM partition chunks
DOF = [0, 128, 256]
TCOL = (BP * T + P - 1) // P  # 2 columns of gathered target rows
EPAD = 384                 # padded row length in fp16 (768B, 256B-aligned)
U_PAD = 16768              # fixed local-table rows (>= 16384+160)

F16 = mybir.dt.float16
I16 = mybir.dt.int16
F32 = mybir.dt.float32

# auxf (f32) column layout
AF_VLOC = 0            # [0:128)   vloc
AF_TLEN = 128          # target_len per partition (p % 32)
AF_T0 = 129            # target t-index for j=0 (p // 32)
AF_T1 = 130            # target t-index for j=1 (4 + p // 32)
AF_ID32 = 131          # [131:163) id32 (rows 0:32)
AF_LINB = 163          # [163:166) lin_b d-chunks
AF_OUTB = 166          # out_b (rows 0:3)
AF_ATTNB = 167         # attn_b (row 0)
AF_CMASK = 168         # [168:296) cmask
AF_CV = 296            # [296:424) cmask * vloc
AF_N = 424

# aux16 (f16) column layout
A6_GSEL = 0            # [0:32)   gsel: chunk c (partition) -> b
A6_SSEL = 32           # [32:64)  ssel: target row p -> b = p % 32
A6_WVEC = 64           # [64:67)  w_vec d-chunks
A6_ONES = 67           # ones column
A6_ONESR = 68          # [68:196) ones row (partition 0)
A6_OUTW = 196          # [196:205) out_w  [d-part, 3 k, C]
A6_N = 205


def _ap2d(tile_ap, col_off, stride, n):
    """2D AP over a 3D tile: partition dim + one strided free dim."""
    return bass.AP(tile_ap.tensor, tile_ap.offset + col_off,
                   [list(tile_ap.ap[0]), [stride, n]])


def _row_rep4(ap2):
    """[1, BP] row AP -> [1, BP, CPB] with the inner dim broadcast (step 0),
    so column c = 4*b + r reads value b."""
    return bass.AP(ap2.tensor, ap2.offset,
                   [list(ap2.ap[0]), [1, BP], [0, CPB]])


DEBUG = False


def build_module():
    nc = bacc.Bacc("TRN2", target_bir_lowering=False, debug=False,
                   num_devices=N_CORES)

    emb_d = nc.dram_tensor("emb_loc", [U_PAD, EPAD], F16, kind="ExternalInput")
    ctx_idx_d = nc.dram_tensor("ctx_idx16", [P, NCH * P // 16], I16,
                               kind="ExternalInput")
    tgt_idx_d = nc.dram_tensor("tgt_idx16", [P, TCOL * P // 16], I16,
                               kind="ExternalInput")
    auxf_d = nc.dram_tensor("auxf_h", [P, AF_N], F32, kind="ExternalInput")
    aux16_d = nc.dram_tensor("aux16_h", [P, A6_N], F16, kind="ExternalInput")
    linw_d = nc.dram_tensor("lin_w_h", [P, 3 * 384], F16, kind="ExternalInput")

    out_d = nc.dram_tensor("logits_t", [C, BP], F32, kind="ExternalOutput")
    if DEBUG:
        dbg_msv_d = nc.dram_tensor("dbg_msv", [P, NCH], F32,
                                   kind="ExternalOutput")
        dbg_em_d = nc.dram_tensor("dbg_em", [P, NCH], F32,
                                  kind="ExternalOutput")
        dbg_va_d = nc.dram_tensor("dbg_va", [BP, D], F32,
                                  kind="ExternalOutput")
        dbg_v1_d = nc.dram_tensor("dbg_v1", [P, 3 * BP], F32,
                                  kind="ExternalOutput")
        dbg_v2_d = nc.dram_tensor("dbg_v2", [P, 3 * BP], F32,
                                  kind="ExternalOutput")

    mult = mybir.AluOpType.mult
    addop = mybir.AluOpType.add
    is_lt = mybir.AluOpType.is_lt
    AF = mybir.ActivationFunctionType

    with tile.TileContext(nc) as tc:
        with (
            tc.tile_pool(name="sb", bufs=1) as sb,
            tc.tile_pool(name="sc", bufs=4) as scr,
            tc.tile_pool(name="ps", bufs=1, space="PSUM") as ps,
            tc.tile_pool(name="ps3", bufs=2, space="PSUM") as ps3,
        ):
            # ---- persistent SBUF tiles ----
            tgti_sb = sb.tile([P, TCOL * P // 16], I16, tag="tgti")
            idx_sb = sb.tile([P, NCH * P // 16], I16, tag="idx")
            auxf_sb = sb.tile([P, AF_N], F32, tag="auxf")
            aux16_sb = sb.tile([P, A6_N], F16, tag="aux16")
            linw_sb = sb.tile([P, 3, 384], F16, tag="linw")
            mem_sb = [sb.tile([P, GW, EPAD], F16, tag=f"mem{g}", name=f"mem{g}")
                      for g in range(NGRP)]
            tgtr_sb = sb.tile([P, TCOL, EPAD], F16, tag="tgtr")

            tmask = sb.tile([P, TCOL], F32, tag="tmask")
            a0 = sb.tile([P, BP, TCOL], F16, tag="a0")
            tlenr = sb.tile([BP, 1], F32, tag="tlenr")
            va_sb = sb.tile([BP, D], F32, tag="va")
            vecT_a = sb.tile([P, 3, BP], F16, tag="vecTa", name="vecT_a")
            vecT_b = sb.tile([P, 3, BP], F16, tag="vecTb", name="vecT_b")
            msv = sb.tile([P, NCH], F32, tag="msv")
            sc_f = sb.tile([P, NCH], F32, tag="scf")
            e_m = sb.tile([P, NCH], F16, tag="em")
            aw = sb.tile([P, NCH], F16, tag="aw")
            aw1 = [sb.tile([P, GW], F16, tag=f"aw1_{g}", name=f"aw1_{g}")
                   for g in range(NGRP)]
            svec_sb = sb.tile([1, BP], F16, tag="svec")
            cs_sb = sb.tile([P, 1], F16, tag="cs")
            rdr_sb = sb.tile([1, BP], F16, tag="rdr")
            rd_sb = sb.tile([P, BP], F32, tag="rdbc_s")
            asm_sb = sb.tile([P, BP], F32, tag="asm")
            lg_sb = sb.tile([C, BP], F32, tag="lg")

            vloc_ap = auxf_sb[:, AF_VLOC:AF_VLOC + NCH]
            cmask_ap = auxf_sb[:, AF_CMASK:AF_CMASK + NCH]
            cv_ap = auxf_sb[:, AF_CV:AF_CV + NCH]
            gsel_ap = aux16_sb[:, A6_GSEL:A6_GSEL + BP]
            ssel_ap = aux16_sb[:, A6_SSEL:A6_SSEL + BP]
            ones_ap = aux16_sb[:, A6_ONES:A6_ONES + 1]
            onesr_ap = aux16_sb[0:1, A6_ONESR:A6_ONESR + P]
            id32_ap = auxf_sb[0:BP, AF_ID32:AF_ID32 + BP]

            # ---- input DMAs (index tensors first so gathers start early) ----
            nc.sync.dma_start(tgti_sb[:], tgt_idx_d.ap())
            nc.sync.dma_start(idx_sb[:], ctx_idx_d.ap())
            nc.sync.dma_start(auxf_sb[:], auxf_d.ap())
            nc.sync.dma_start(aux16_sb[:], aux16_d.ap())
            nc.sync.dma_start(linw_sb[:], linw_d.ap())

            # ---- gathers ----
            nc.gpsimd.dma_gather(
                out_ap=tgtr_sb[:], in_ap=emb_d.ap(), idxs_ap=tgti_sb[:],
                num_idxs=TCOL * P, num_idxs_reg=TCOL * P, elem_size=EPAD)
            NIG = GW * P  # idxs per gather group
            for g in range(NGRP):
                nc.gpsimd.dma_gather(
                    out_ap=mem_sb[g][:], in_ap=emb_d.ap(),
                    idxs_ap=idx_sb[:, g * (NIG // 16):(g + 1) * (NIG // 16)],
                    num_idxs=NIG, num_idxs_reg=NIG, elem_size=EPAD)

            # ---- v_aspect -> vecT_a ----
            nc.vector.tensor_tensor(
                out=tmask[:], in0=auxf_sb[:, AF_T0:AF_T0 + TCOL],
                in1=auxf_sb[:, AF_TLEN:AF_TLEN + 1].to_broadcast([P, TCOL]),
                op=is_lt)
            va_ps = ps.tile([BP, D], F32, tag="acc300", space="PSUM")
            for j in range(TCOL):
                nc.vector.tensor_scalar_mul(a0[:, :, j], ssel_ap,
                                            tmask[:, j:j + 1])
                nc.tensor.matmul(va_ps[:], lhsT=a0[:, :, j],
                                 rhs=tgtr_sb[:, j, 0:D],
                                 start=(j == 0), stop=(j == TCOL - 1))
            nc.vector.reciprocal(tlenr[:], auxf_sb[0:BP, AF_TLEN:AF_TLEN + 1])
            nc.vector.tensor_scalar_mul(va_sb[:], va_ps[:], tlenr[:])
            for k in range(3):
                kk = DK[k]
                t_ps = ps3.tile([P, BP], F32, tag="psmall", space="PSUM")
                nc.tensor.transpose(t_ps[:kk, :], va_sb[:, DOF[k]:DOF[k] + kk],
                                    id32_ap)
                nc.vector.tensor_copy(out=vecT_a[:kk, k, :], in_=t_ps[:kk, :])

            # PSUM accumulation semantics: start=True lazily zeroes the whole
            # 2KB bank (each byte is overwritten by its first write after the
            # start). So each PSUM tile gets exactly ONE start (first matmul)
            # and ONE stop (last matmul); disjoint sub-regions accumulate
            # independently in between.
            def lin_mms(vcur, lin_ps):
                for k in (0, 2, 1):  # 128-partition slice first and last
                    kk = DK[k]
                    for kx in range(3):
                        kkx = DK[kx]
                        nc.tensor.matmul(
                            lin_ps[:kk, k, :],
                            lhsT=linw_sb[:kkx, kx, DOF[k]:DOF[k] + kk],
                            rhs=vcur[:kkx, kx, :],
                            start=(k == 0 and kx == 0),
                            stop=(k == 1 and kx == 2))

            def svec_bc(vcur):
                """svec = vec @ w_vec + attn_b, broadcast to [P, NCH]."""
                svec_ps = ps3.tile([1, BP], F32, tag="psmall", space="PSUM")
                for k in range(3):
                    kk = DK[k]
                    nc.tensor.matmul(svec_ps[:],
                                     lhsT=aux16_sb[:kk, A6_WVEC + k:A6_WVEC + k + 1],
                                     rhs=vcur[:kk, k, :],
                                     start=(k == 0), stop=(k == 2))
                nc.vector.tensor_scalar_add(svec_sb[:], svec_ps[:],
                                            auxf_sb[0:1, AF_ATTNB:AF_ATTNB + 1])
                svbc_ps = ps.tile([P, NCH], F32, tag="svbc", space="PSUM")
                nc.tensor.matmul(svbc_ps[:], lhsT=onesr_ap,
                                 rhs=_row_rep4(svec_sb[:]),
                                 start=True, stop=True)
                return svbc_ps

            def attn_mms(attn_ps, alpha_col, c):
                """3 accumulating matmuls: attn^T[:, b] += mem_c^T-slices @ alpha.

                The whole [P, 3, BP] tile is one PSUM group per hop: start on
                the very first matmul, stop on the very last (see note above).
                """
                g, cc = divmod(c, GW)
                b, r = divmod(c, CPB)
                # k order (0, 2, 1): the group's first AND last matmul must
                # span all 128 partitions (start/stop flag the bank on the
                # instruction's own partition range only)
                for k in (0, 2, 1):
                    kk = DK[k]
                    nc.tensor.matmul(
                        attn_ps[:kk, k, b:b + 1],
                        lhsT=mem_sb[g][:, cc, DOF[k]:DOF[k] + DK[k]],
                        rhs=alpha_col,
                        start=(c == 0 and k == 0),
                        stop=(c == NCH - 1 and k == 1))

            def denom_tail(dn_ps):
                """reciprocal + broadcast of the softmax denominator."""
                with nc.allow_low_precision(reason="fp16 1/denom, rel 5e-4"):
                    nc.vector.reciprocal(rdr_sb[:], dn_ps[:])
                rd_ps = ps3.tile([P, BP], F32, tag="psmall", space="PSUM")
                nc.tensor.matmul(rd_ps[:], lhsT=onesr_ap, rhs=rdr_sb[:],
                                 start=True, stop=True)
                # HW: a DVE op may read at most one PSUM operand; stage the
                # broadcast denominator in SBUF for the assemble ops
                nc.vector.tensor_copy(out=rd_sb[:], in_=rd_ps[:])
                return rd_sb

            def assemble(attn_ps, rd_bc, lin_ps, vnxt):
                # k=1 first: its PSUM reads depend on both groups' stop
                # matmuls (the k=1 sections), so the later k=2/k=0 reads on
                # the in-order DVE are also past them
                for k in (1, 2, 0):
                    kk = DK[k]
                    nc.vector.tensor_tensor(out=asm_sb[:kk, :],
                                            in0=attn_ps[:kk, k, :],
                                            in1=rd_bc[:kk, :], op=mult)
                    nc.vector.scalar_tensor_tensor(
                        out=vnxt[:kk, k, :], in0=lin_ps[:kk, k, :],
                        scalar=auxf_sb[:kk, AF_LINB + k:AF_LINB + k + 1],
                        in1=asm_sb[:kk, :], op0=addop, op1=addop)

            # ======== hop 1 (pipelined per gather group) ========
            lin_ps = ps.tile([P, 3, BP], F32, tag="accL", space="PSUM")
            lin_mms(vecT_a, lin_ps)
            svbc_ps = svec_bc(vecT_a)
            attn_ps = ps.tile([P, 3, BP], F32, tag="accA", space="PSUM")
            for g in range(NGRP):
                gs = g * GW
                score_ap = _ap2d(mem_sb[g][:], D, EPAD, GW)
                nc.vector.tensor_tensor(out=msv[:, gs:gs + GW], in0=score_ap,
                                        in1=vloc_ap[:, gs:gs + GW], op=mult)
                st = scr.tile([P, GW], F32, tag="st", bufs=4)
                nc.vector.tensor_tensor(out=st[:], in0=msv[:, gs:gs + GW],
                                        in1=svbc_ps[:, gs:gs + GW], op=addop)
                nc.scalar.activation(st[:], st[:], AF.Tanh)
                nc.scalar.activation(st[:], st[:], AF.Exp)
                nc.vector.tensor_tensor(out=e_m[:, gs:gs + GW], in0=st[:],
                                        in1=cmask_ap[:, gs:gs + GW], op=mult)
                nc.vector.tensor_tensor(out=aw1[g][:], in0=st[:],
                                        in1=cv_ap[:, gs:gs + GW], op=mult)
                for cc in range(GW):
                    attn_mms(attn_ps, aw1[g][:, cc:cc + 1], gs + cc)
            cs_ps = ps3.tile([P, 1], F32, tag="psmall", space="PSUM")
            nc.tensor.matmul(cs_ps[:], lhsT=e_m[:], rhs=ones_ap,
                             start=True, stop=True)
            nc.vector.tensor_copy(out=cs_sb[:], in_=cs_ps[:])
            dn_ps = ps3.tile([1, BP], F32, tag="psmall", space="PSUM")
            nc.tensor.matmul(dn_ps[:], lhsT=cs_sb[:], rhs=gsel_ap,
                             start=True, stop=True)
            rd_ps = denom_tail(dn_ps)
            assemble(attn_ps, rd_ps, lin_ps, vecT_b)

            if DEBUG:
                dbg_va = sb.tile([BP, D], F32, tag="dbg_va_t")
                nc.vector.tensor_copy(out=dbg_va[:], in_=va_sb[:])
                nc.sync.dma_start(dbg_va_d.ap(), dbg_va[:])
                dbg_msv = sb.tile([P, NCH], F32, tag="dbg_msv_t")
                nc.vector.tensor_copy(out=dbg_msv[:], in_=msv[:])
                nc.sync.dma_start(dbg_msv_d.ap(), dbg_msv[:])
                dbg_em = sb.tile([P, NCH], F32, tag="dbg_em_t")
                nc.vector.tensor_copy(out=dbg_em[:], in_=e_m[:])
                nc.sync.dma_start(dbg_em_d.ap(), dbg_em[:])
                dbg_v1 = sb.tile([P, 3, BP], F32, tag="dbg_v1_t")
                nc.vector.memset(dbg_v1[:], 0.0)
                for k in range(3):
                    nc.vector.tensor_copy(out=dbg_v1[:DK[k], k, :],
                                          in_=vecT_b[:DK[k], k, :])
                nc.sync.dma_start(dbg_v1_d.ap(), dbg_v1[:])

            # ======== hops 2..N ========
            for h in range(1, N_HOPS):
                vcur = vecT_b if h % 2 == 1 else vecT_a
                vnxt = vecT_a if h % 2 == 1 else vecT_b
                lin_ps = ps.tile([P, 3, BP], F32, tag="accL", space="PSUM")
                lin_mms(vcur, lin_ps)
                svbc_ps = svec_bc(vcur)
                nc.vector.tensor_tensor(out=sc_f[:], in0=msv[:],
                                        in1=svbc_ps[:], op=addop)
                nc.scalar.activation(sc_f[:], sc_f[:], AF.Tanh)
                nc.scalar.activation(sc_f[:], sc_f[:], AF.Exp)
                nc.vector.tensor_tensor(out=e_m[:], in0=sc_f[:],
                                        in1=cmask_ap, op=mult)
                nc.vector.tensor_tensor(out=aw[:], in0=sc_f[:],
                                        in1=cv_ap, op=mult)
                attn_ps = ps.tile([P, 3, BP], F32, tag="accA", space="PSUM")
                cs_ps = ps3.tile([P, 1], F32, tag="psmall", space="PSUM")
                dn_ps = ps3.tile([1, BP], F32, tag="psmall", space="PSUM")
                # denominator ops interleave with the attention stream so the
                # PE never stalls on the DVE copy/reciprocal chain
                nc.tensor.matmul(cs_ps[:], lhsT=e_m[:], rhs=ones_ap,
                                 start=True, stop=True)
                nc.vector.tensor_copy(out=cs_sb[:], in_=cs_ps[:])
                for c in range(NCH // 3):
                    attn_mms(attn_ps, aw[:, c:c + 1], c)
                nc.tensor.matmul(dn_ps[:], lhsT=cs_sb[:], rhs=gsel_ap,
                                 start=True, stop=True)
                for c in range(NCH // 3, 2 * NCH // 3):
                    attn_mms(attn_ps, aw[:, c:c + 1], c)
                rd_ps = denom_tail(dn_ps)
                for c in range(2 * NCH // 3, NCH):
                    attn_mms(attn_ps, aw[:, c:c + 1], c)
                assemble(attn_ps, rd_ps, lin_ps, vnxt)
                if DEBUG and h == 1:
                    dbg_v2 = sb.tile([P, 3, BP], F32, tag="dbg_v2_t")
                    nc.vector.memset(dbg_v2[:], 0.0)
                    for k in range(3):
                        nc.vector.tensor_copy(out=dbg_v2[:DK[k], k, :],
                                              in_=vnxt[:DK[k], k, :])
                    nc.sync.dma_start(dbg_v2_d.ap(), dbg_v2[:])

            # ---- output projection ----
            vfin = vecT_b if N_HOPS % 2 == 1 else vecT_a
            lg_ps = ps3.tile([C, BP], F32, tag="psmall", space="PSUM")
            for k in range(3):
                kk = DK[k]
                nc.tensor.matmul(
                    lg_ps[:],
                    lhsT=aux16_sb[:kk, A6_OUTW + k * C:A6_OUTW + (k + 1) * C],
                    rhs=vfin[:kk, k, :], start=(k == 0), stop=(k == 2))
            nc.vector.tensor_scalar_add(lg_sb[:], lg_ps[:],
                                        auxf_sb[0:C, AF_OUTB:AF_OUTB + 1])
            nc.sync.dma_start(out_d.ap(), lg_sb[:])

    nc.compile()
    return nc


def _wrap16(flat):
    """dma_gather index layout: [128, n/16], replicated over 16-row groups."""
    n = flat.shape[0]
    w = flat.reshape(n // 16, 16).T.astype(np.int16)   # [16, n/16]
    return np.ascontiguousarray(np.tile(w, (8, 1)))    # [128, n/16]


def make_core_inputs(context_x, context_len, target_x, target_len, target_loc,
                     emb16, shared):
    """Per-core input dict. context_x etc are the 32-row shards (numpy).

    The embedding table is sharded per core by index compaction: each core
    receives only the (unique) rows its shard references, padded to 384
    columns (768B, a dma_gather-legal element size) with the precomputed
    content score emb@w_mem at column 300, plus int16 local indices in the
    wrapped dma_gather layout.
    """
    score16 = shared["_score16"]
    flat = np.ascontiguousarray(context_x, dtype=np.int64).reshape(-1)
    tflat = np.zeros(P * TCOL, np.int64)
    tflat[:BP * T] = np.ascontiguousarray(target_x.T, dtype=np.int64).reshape(-1)
    allidx = np.concatenate([flat, tflat])
    uniq, inv = np.unique(allidx, return_inverse=True)
    assert uniq.shape[0] <= U_PAD
    emb_loc = np.zeros((U_PAD, EPAD), np.float16)
    emb_loc[:uniq.shape[0], :D] = emb16[uniq]
    emb_loc[:uniq.shape[0], D] = score16[uniq]
    ctx_idx = _wrap16(inv[:flat.shape[0]])
    tgt_idx = _wrap16(inv[flat.shape[0]:])

    # host-side location model per (p, c): b = c//4, l = (c%4)*128 + p
    cidx = np.arange(NCH) // CPB
    pos = ((np.arange(NCH)[None, :] % CPB) * P
           + np.arange(P)[:, None]).astype(np.float64)
    loc_b = target_loc[cidx].astype(np.float64)[None, :]
    len_b = context_len[cidx].astype(np.float64)[None, :]
    vloc = 1.0 - np.abs(pos - loc_b) / len_b
    cmask = (pos < len_b).astype(np.float64)

    auxf = np.zeros((P, AF_N), np.float32)
    auxf[:, AF_VLOC:AF_VLOC + NCH] = vloc
    auxf[:, AF_TLEN] = target_len[np.arange(P) % BP]
    auxf[:, AF_T0] = np.arange(P) // BP
    auxf[:, AF_T1] = (P // BP) + np.arange(P) // BP
    auxf[:BP, AF_ID32:AF_ID32 + BP] = np.eye(BP)
    auxf[:, AF_LINB:AF_LINB + 3] = shared["_linb3"]
    auxf[:C, AF_OUTB] = shared["_outb"]
    auxf[0, AF_ATTNB] = shared["_attnb"]
    auxf[:, AF_CMASK:AF_CMASK + NCH] = cmask
    auxf[:, AF_CV:AF_CV + NCH] = cmask * vloc

    d = dict(aux16_h=shared["aux16_h"], lin_w_h=shared["lin_w_h"])
    d.update(emb_loc=emb_loc, ctx_idx16=ctx_idx, tgt_idx16=tgt_idx,
             auxf_h=auxf)
    return d


def make_shared_inputs(emb, attn_w, attn_b, lin_w, lin_b, out_w, out_b):
    lin_w_pad = np.zeros((384, 384), np.float16)
    lin_w_pad[:D, :D] = lin_w.astype(np.float16)
    lin_w_h = np.ascontiguousarray(
        lin_w_pad.reshape(3, P, 384).transpose(1, 0, 2).reshape(P, 3 * 384))

    aux16 = np.zeros((P, A6_N), np.float16)
    aux16[:, A6_GSEL:A6_GSEL + BP] = (
        np.arange(P)[:, None] // CPB == np.arange(BP)[None, :])
    aux16[:, A6_SSEL:A6_SSEL + BP] = (
        np.arange(P)[:, None] % BP == np.arange(BP)[None, :])
    w_vec_pad = np.zeros((384,), np.float16)
    w_vec_pad[:D] = attn_w[D:, 0].astype(np.float16)
    aux16[:, A6_WVEC:A6_WVEC + 3] = w_vec_pad.reshape(3, P).T
    aux16[:, A6_ONES] = 1.0
    aux16[0, A6_ONESR:A6_ONESR + P] = 1.0
    out_w_pad = np.zeros((384, C), np.float16)
    out_w_pad[:D] = out_w.astype(np.float16)
    aux16[:, A6_OUTW:A6_OUTW + 3 * C] = (
        out_w_pad.reshape(3, P, C).transpose(1, 0, 2).reshape(P, 3 * C))

    lin_b_pad = np.zeros((384,), np.float32)
    lin_b_pad[:D] = lin_b
    score16 = (np.asarray(emb, np.float64)
               @ np.asarray(attn_w[:D, 0], np.float64)).astype(np.float16)
    return dict(
        lin_w_h=lin_w_h,
        aux16_h=aux16,
        _linb3=np.ascontiguousarray(lin_b_pad.reshape(3, P).T),
        _outb=out_b.astype(np.float32),
        _attnb=np.float32(attn_b[0]),
        _score16=score16,
    )


_module_cache = {}


def get_module():
    if "nc" not in _module_cache:
        _module_cache["nc"] = build_module()
    return _module_cache["nc"]


def kernel(**inputs):
    emb16 = np.ascontiguousarray(inputs["emb"].astype(np.float16))
    shared = make_shared_inputs(
        np.asarray(inputs["emb"]), np.asarray(inputs["attn_w"]),
        np.asarray(inputs["attn_b"]), np.asarray(inputs["lin_w"]),
        np.asarray(inputs["lin_b"]), np.asarray(inputs["out_w"]),
        np.asarray(inputs["out_b"]))
    in_maps = []
    for k in range(N_CORES):
        s = slice(k * BP, (k + 1) * BP)
        in_maps.append(make_core_inputs(
            np.asarray(inputs["context_x"])[s],
            np.asarray(inputs["context_len"])[s],
            np.asarray(inputs["target_x"])[s],
            np.asarray(inputs["target_len"])[s],
            np.asarray(inputs["target_loc"])[s],
            emb16, shared))
    nc = get_module()
    res = bass_utils.run_bass_kernel_spmd(nc, in_maps,
                                          core_ids=list(range(N_CORES)))
    out = np.concatenate([res.results[k]["logits_t"].T
                          for k in range(N_CORES)], axis=0)
    return out.astype(np.float32)


# revision 40
# speedup vs baseline: 1.0332x; 1.0332x over previous
"""MemNet (scatter_memory) Trainium2 kernel, v2.

Model (per batch row b):
  memory   = emb[context_x[b]]                    # [L, D] gather
  v_aspect = masked-mean(emb[target_x[b]])        # [D]
  v_loc    = 1 - |pos - target_loc[b]| / context_len[b]
  3 hops of: scores = tanh((memory*v_loc) @ w_mem + vec@w_vec + b)
             alpha  = masked softmax;  vec = alpha @ (memory*v_loc) + vec@lin_w+lin_b
  logits   = vec @ out_w + out_b

Sharding: data-parallel over batch, 32 rows per core on 8 cores; the fp16
embedding table is index-compacted per core and fetched by indirect DMA
gather.

Key structure (vs v1):
- The content score emb@w_mem is a pure weight transform; it is appended
  as column 300 of the gathered rows (inside the 768B alignment padding),
  so scores arrive with the gather for free.
- Attention contraction runs transposed on the PE: per (chunk, d-slice),
  stationary = mem[128, DK], moving = the chunk's alpha column [128, 1],
  accumulating over each batch row's 4 chunks directly into vec^T layout
  [DK, b] in PSUM. Cost ~ 1 cycle per matmul (output free size 1).
- v_loc/cmask are host-side index-derived tensors; hop 1 (whose alpha
  depends only on v_aspect) is pipelined per gather group so its attention
  runs under the gather phase; the softmax denominator accumulates per
  group in an open PSUM matmul group.
"""

import numpy as np

import concourse.bass as bass
import concourse.bacc as bacc
import concourse.mybir as mybir
import concourse.tile as tile
from concourse import bass_utils

N_CORES = 8
B, L, T, V, D, C = 256, 512, 5, 50000, 300, 3
N_HOPS = 3
BP = B // N_CORES          # 32 batch rows per core
P = 128                    # partitions
NCH = (BP * L) // P        # 128 chunk columns; chunk c: b=c//4, l=(c%4)*128+p
CPB = L // P               # 4 chunks per batch row
NGRP = 16                  # gather groups (<=1024 idxs per dma_gather)
GW = NCH // NGRP           # chunk columns per gather group (8)
DK = [128, 128, 44]        # D split across PSUM partition chunks
DOF = [0, 128, 256]
TCOL = (BP * T + P - 1) // P  # 2 columns of host-provided target rows
EPAD16 = 384               # row length in f16 units (768B rows: 300 fp16
                           # values + fp16 content score, 256B-aligned)
SCORE_COL = 300            # f16 column of the packed content score
TE = 304                   # target row pad (f16 units)
U_PAD = 16768              # fixed local-table rows (>= 16384)

F16 = mybir.dt.float16
I16 = mybir.dt.int16
F32 = mybir.dt.float32
F8 = mybir.dt.float8e4

# auxf (f32) column layout
AF_VLOC = 0            # [0:128)   vloc
AF_TLEN = 128          # target_len per partition (p % 32)
AF_T0 = 129            # target t-index for j=0 (p // 32)
AF_T1 = 130            # target t-index for j=1 (4 + p // 32)
AF_ID32 = 131          # [131:163) id32 (rows 0:32)
AF_LINB = 163          # [163:166) lin_b d-chunks
AF_OUTB = 166          # out_b (rows 0:3)
AF_ATTNB = 167         # attn_b (row 0)
AF_CMASK = 168         # [168:296) cmask
AF_CV = 296            # [296:424) cmask * vloc
AF_N = 424

# aux16 (f16) column layout
A6_GSEL = 0            # [0:32)   gsel: chunk c (partition) -> b
A6_SSEL = 32           # [32:64)  ssel: target row p -> b = p % 32
A6_WVEC = 64           # [64:67)  w_vec d-chunks
A6_ONES = 67           # ones column
A6_ONESR = 68          # [68:196) ones row (partition 0)
A6_OUTW = 196          # [196:205) out_w  [d-part, 3 k, C]
A6_N = 205


def _ap2d(tile_ap, col_off, stride, n):
    """2D AP over a 3D tile: partition dim + one strided free dim."""
    return bass.AP(tile_ap.tensor, tile_ap.offset + col_off,
                   [list(tile_ap.ap[0]), [stride, n]])


def _row_rep4(ap2):
    """[1, BP] row AP -> [1, BP, CPB] with the inner dim broadcast (step 0),
    so column c = 4*b + r reads value b."""
    return bass.AP(ap2.tensor, ap2.offset,
                   [list(ap2.ap[0]), [1, BP], [0, CPB]])


DEBUG = False


def build_module():
    nc = bacc.Bacc("TRN2", target_bir_lowering=False, debug=False,
                   num_devices=N_CORES)

    emb_d = nc.dram_tensor("emb_loc", [U_PAD, EPAD16], F16,
                           kind="ExternalInput")
    ctx_idx_d = nc.dram_tensor("ctx_idx16", [P, NCH * P // 16], I16,
                               kind="ExternalInput")
    tgtr_d = nc.dram_tensor("tgtr_h", [P, TCOL * TE], F16,
                            kind="ExternalInput")
    auxf_d = nc.dram_tensor("auxf_h", [P, AF_N], F32, kind="ExternalInput")
    aux16_d = nc.dram_tensor("aux16_h", [P, A6_N], F16, kind="ExternalInput")
    linw_d = nc.dram_tensor("lin_w_h", [P, 3 * 384], F16, kind="ExternalInput")

    out_d = nc.dram_tensor("logits_t", [C, BP], F32, kind="ExternalOutput")
    if DEBUG:
        dbg_msv_d = nc.dram_tensor("dbg_msv", [P, NCH], F32,
                                   kind="ExternalOutput")
        dbg_em_d = nc.dram_tensor("dbg_em", [P, NCH], F32,
                                  kind="ExternalOutput")
        dbg_va_d = nc.dram_tensor("dbg_va", [BP, D], F32,
                                  kind="ExternalOutput")
        dbg_v1_d = nc.dram_tensor("dbg_v1", [P, 3 * BP], F32,
                                  kind="ExternalOutput")
        dbg_v2_d = nc.dram_tensor("dbg_v2", [P, 3 * BP], F32,
                                  kind="ExternalOutput")

    mult = mybir.AluOpType.mult
    addop = mybir.AluOpType.add
    is_lt = mybir.AluOpType.is_lt
    AF = mybir.ActivationFunctionType

    with tile.TileContext(nc) as tc:
        with (
            tc.tile_pool(name="sb", bufs=1) as sb,
            tc.tile_pool(name="sc", bufs=4) as scr,
            tc.tile_pool(name="ps", bufs=1, space="PSUM") as ps,
            tc.tile_pool(name="ps3", bufs=2, space="PSUM") as ps3,
        ):
            # ---- persistent SBUF tiles ----
            idx_sb = sb.tile([P, NCH * P // 16], I16, tag="idx")
            auxf_sb = sb.tile([P, AF_N], F32, tag="auxf")
            aux16_sb = sb.tile([P, A6_N], F16, tag="aux16")
            linw_sb = sb.tile([P, 3, 384], F16, tag="linw")
            mem_sb = [sb.tile([P, GW, EPAD16], F16, tag=f"mem{g}",
                              name=f"mem{g}") for g in range(NGRP)]
            tgtr_sb = sb.tile([P, TCOL, TE], F16, tag="tgtr")

            tmask = sb.tile([P, TCOL], F32, tag="tmask")
            a0 = sb.tile([P, BP, TCOL], F16, tag="a0")
            tlenr = sb.tile([BP, 1], F32, tag="tlenr")
            va_sb = sb.tile([BP, D], F32, tag="va")
            vecT_a = sb.tile([P, 3, BP], F16, tag="vecTa", name="vecT_a")
            vecT_b = sb.tile([P, 3, BP], F16, tag="vecTb", name="vecT_b")
            msv = sb.tile([P, NCH], F32, tag="msv")
            sc_f = sb.tile([P, NCH], F32, tag="scf")
            e_m = sb.tile([P, NCH], F16, tag="em")
            aw = sb.tile([P, NCH], F16, tag="aw")
            aw1 = [sb.tile([P, GW], F16, tag=f"aw1_{g}", name=f"aw1_{g}")
                   for g in range(NGRP)]
            svec_sb = sb.tile([1, BP], F16, tag="svec")
            cs_sb = sb.tile([P, 1], F16, tag="cs")
            rdr_sb = sb.tile([1, BP], F16, tag="rdr")
            rd_sb = sb.tile([P, BP], F32, tag="rdbc_s")
            asm3 = sb.tile([P, 3, BP], F32, tag="asm")
            lg_sb = sb.tile([C, BP], F32, tag="lg")

            vloc_ap = auxf_sb[:, AF_VLOC:AF_VLOC + NCH]
            cmask_ap = auxf_sb[:, AF_CMASK:AF_CMASK + NCH]
            cv_ap = auxf_sb[:, AF_CV:AF_CV + NCH]
            gsel_ap = aux16_sb[:, A6_GSEL:A6_GSEL + BP]
            ssel_ap = aux16_sb[:, A6_SSEL:A6_SSEL + BP]
            ones_ap = aux16_sb[:, A6_ONES:A6_ONES + 1]
            onesr_ap = aux16_sb[0:1, A6_ONESR:A6_ONESR + P]
            id32_ap = auxf_sb[0:BP, AF_ID32:AF_ID32 + BP]

            # ---- input DMAs (context indices first so gathers start early) ----
            nc.sync.dma_start(idx_sb[:], ctx_idx_d.ap())
            nc.sync.dma_start(tgtr_sb[:], tgtr_d.ap())
            nc.sync.dma_start(auxf_sb[:], auxf_d.ap())
            nc.sync.dma_start(aux16_sb[:], aux16_d.ap())
            nc.sync.dma_start(linw_sb[:], linw_d.ap())

            # ---- gathers (512B rows: fp8 payload + fp16 score) ----
            NIG = GW * P  # idxs per gather group
            for g in range(NGRP):
                nc.gpsimd.dma_gather(
                    out_ap=mem_sb[g][:], in_ap=emb_d.ap(),
                    idxs_ap=idx_sb[:, g * (NIG // 16):(g + 1) * (NIG // 16)],
                    num_idxs=NIG, num_idxs_reg=NIG, elem_size=EPAD16)

            # ---- v_aspect -> vecT_a ----
            nc.vector.tensor_tensor(
                out=tmask[:], in0=auxf_sb[:, AF_T0:AF_T0 + TCOL],
                in1=auxf_sb[:, AF_TLEN:AF_TLEN + 1].to_broadcast([P, TCOL]),
                op=is_lt)
            va_ps = ps.tile([BP, D], F32, tag="acc300", space="PSUM")
            for j in range(TCOL):
                nc.vector.tensor_scalar_mul(a0[:, :, j], ssel_ap,
                                            tmask[:, j:j + 1])
                nc.tensor.matmul(va_ps[:], lhsT=a0[:, :, j],
                                 rhs=tgtr_sb[:, j, 0:D],
                                 start=(j == 0), stop=(j == TCOL - 1))
            nc.vector.reciprocal(tlenr[:], auxf_sb[0:BP, AF_TLEN:AF_TLEN + 1])
            nc.vector.tensor_scalar_mul(va_sb[:], va_ps[:], tlenr[:])
            for k in range(3):
                kk = DK[k]
                t_ps = ps3.tile([P, BP], F32, tag="psmall", space="PSUM")
                nc.tensor.transpose(t_ps[:kk, :], va_sb[:, DOF[k]:DOF[k] + kk],
                                    id32_ap)
                nc.vector.tensor_copy(out=vecT_a[:kk, k, :], in_=t_ps[:kk, :])

            # PSUM accumulation semantics: start=True lazily zeroes the whole
            # 2KB bank (each byte is overwritten by its first write after the
            # start). So each PSUM tile gets exactly ONE start (first matmul)
            # and ONE stop (last matmul); disjoint sub-regions accumulate
            # independently in between.
            def lin_mms(vcur, lin_ps):
                for k in (0, 2, 1):  # 128-partition slice first and last
                    kk = DK[k]
                    for kx in range(3):
                        kkx = DK[kx]
                        nc.tensor.matmul(
                            lin_ps[:kk, k, :],
                            lhsT=linw_sb[:kkx, kx, DOF[k]:DOF[k] + kk],
                            rhs=vcur[:kkx, kx, :],
                            start=(k == 0 and kx == 0),
                            stop=(k == 1 and kx == 2))

            def svec_bc(vcur):
                """svec = vec @ w_vec + attn_b, broadcast to [P, NCH]."""
                svec_ps = ps3.tile([1, BP], F32, tag="psmall", space="PSUM")
                for k in range(3):
                    kk = DK[k]
                    nc.tensor.matmul(svec_ps[:],
                                     lhsT=aux16_sb[:kk, A6_WVEC + k:A6_WVEC + k + 1],
                                     rhs=vcur[:kk, k, :],
                                     start=(k == 0), stop=(k == 2))
                nc.vector.tensor_scalar_add(svec_sb[:], svec_ps[:],
                                            auxf_sb[0:1, AF_ATTNB:AF_ATTNB + 1])
                svbc_ps = ps.tile([P, NCH], F32, tag="svbc", space="PSUM")
                nc.tensor.matmul(svbc_ps[:], lhsT=onesr_ap,
                                 rhs=_row_rep4(svec_sb[:]),
                                 start=True, stop=True)
                return svbc_ps

            def attn_mms(attn_ps, alpha_col, c):
                """3 accumulating matmuls: attn^T[:, b] += mem_c^T-slices @ alpha.

                The whole [P, 3, BP] tile is one PSUM group per hop: start on
                the very first matmul, stop on the very last (see note above).
                """
                g, cc = divmod(c, GW)
                b, r = divmod(c, CPB)
                # k order (0, 2, 1): the group's first AND last matmul must
                # span all 128 partitions (start/stop flag the bank on the
                # instruction's own partition range only)
                for k in (0, 2, 1):
                    kk = DK[k]
                    nc.tensor.matmul(
                        attn_ps[:kk, k, b:b + 1],
                        lhsT=mem_sb[g][:, cc, DOF[k]:DOF[k] + DK[k]],
                        rhs=alpha_col,
                        start=(c == 0 and k == 0),
                        stop=(c == NCH - 1 and k == 1))

            def denom_recip(dn_ps):
                """reciprocal of the softmax denominator row."""
                with nc.allow_low_precision(reason="fp16 1/denom, rel 5e-4"):
                    nc.vector.reciprocal(rdr_sb[:], dn_ps[:])

            def denom_bcast():
                """broadcast 1/denom across partitions, staged to SBUF (HW: a
                DVE op may read at most one PSUM operand)."""
                rd_ps = ps3.tile([P, BP], F32, tag="psmall", space="PSUM")
                nc.tensor.matmul(rd_ps[:], lhsT=onesr_ap, rhs=rdr_sb[:],
                                 start=True, stop=True)
                nc.vector.tensor_copy(out=rd_sb[:], in_=rd_ps[:])
                return rd_sb

            def assemble(attn_ps, rd_bc, lin_ps, vnxt):
                # k=1 first: its PSUM reads depend on both groups' stop
                # matmuls (the k=1 sections), so the later k=2/k=0 reads on
                # the in-order DVE are also past them. Phase A ops are
                # mutually independent; each phase-B op depends on a phase-A
                # op 3 slots back, so the RAW sem is already satisfied.
                for k in (1, 2, 0):
                    kk = DK[k]
                    nc.vector.tensor_tensor(out=asm3[:kk, k, :],
                                            in0=attn_ps[:kk, k, :],
                                            in1=rd_bc[:kk, :], op=mult)
                for k in (1, 2, 0):
                    kk = DK[k]
                    nc.vector.scalar_tensor_tensor(
                        out=vnxt[:kk, k, :], in0=lin_ps[:kk, k, :],
                        scalar=auxf_sb[:kk, AF_LINB + k:AF_LINB + k + 1],
                        in1=asm3[:kk, k, :], op0=addop, op1=addop)

            # ======== hop 1 (pipelined per gather group) ========
            lin_ps = ps.tile([P, 3, BP], F32, tag="accL", space="PSUM")
            lin_mms(vecT_a, lin_ps)
            svbc_ps = svec_bc(vecT_a)
            attn_ps = ps.tile([P, 3, BP], F32, tag="accA", space="PSUM")
            for g in range(NGRP):
                gs = g * GW
                score_ap = _ap2d(mem_sb[g][:], SCORE_COL, EPAD16, GW)
                nc.vector.tensor_tensor(out=msv[:, gs:gs + GW], in0=score_ap,
                                        in1=vloc_ap[:, gs:gs + GW], op=mult)
                st = scr.tile([P, GW], F32, tag="st", bufs=4)
                nc.vector.tensor_tensor(out=st[:], in0=msv[:, gs:gs + GW],
                                        in1=svbc_ps[:, gs:gs + GW], op=addop)
                nc.scalar.activation(st[:], st[:], AF.Tanh)
                nc.scalar.activation(st[:], st[:], AF.Exp)
                nc.vector.tensor_tensor(out=e_m[:, gs:gs + GW], in0=st[:],
                                        in1=cmask_ap[:, gs:gs + GW], op=mult)
                nc.vector.tensor_tensor(out=aw1[g][:], in0=st[:],
                                        in1=cv_ap[:, gs:gs + GW], op=mult)
                for cc in range(GW):
                    attn_mms(attn_ps, aw1[g][:, cc:cc + 1], gs + cc)
            cs_ps = ps3.tile([P, 1], F32, tag="psmall", space="PSUM")
            nc.tensor.matmul(cs_ps[:], lhsT=e_m[:], rhs=ones_ap,
                             start=True, stop=True)
            nc.vector.tensor_copy(out=cs_sb[:], in_=cs_ps[:])
            dn_ps = ps3.tile([1, BP], F32, tag="psmall", space="PSUM")
            nc.tensor.matmul(dn_ps[:], lhsT=cs_sb[:], rhs=gsel_ap,
                             start=True, stop=True)
            denom_recip(dn_ps)
            rd_bc = denom_bcast()
            assemble(attn_ps, rd_bc, lin_ps, vecT_b)

            if DEBUG:
                dbg_va = sb.tile([BP, D], F32, tag="dbg_va_t")
                nc.vector.tensor_copy(out=dbg_va[:], in_=va_sb[:])
                nc.sync.dma_start(dbg_va_d.ap(), dbg_va[:])
                dbg_msv = sb.tile([P, NCH], F32, tag="dbg_msv_t")
                nc.vector.tensor_copy(out=dbg_msv[:], in_=msv[:])
                nc.sync.dma_start(dbg_msv_d.ap(), dbg_msv[:])
                dbg_em = sb.tile([P, NCH], F32, tag="dbg_em_t")
                nc.vector.tensor_copy(out=dbg_em[:], in_=e_m[:])
                nc.sync.dma_start(dbg_em_d.ap(), dbg_em[:])
                dbg_v1 = sb.tile([P, 3, BP], F32, tag="dbg_v1_t")
                nc.vector.memset(dbg_v1[:], 0.0)
                for k in range(3):
                    nc.vector.tensor_copy(out=dbg_v1[:DK[k], k, :],
                                          in_=vecT_b[:DK[k], k, :])
                nc.sync.dma_start(dbg_v1_d.ap(), dbg_v1[:])

            # ======== hops 2..N ========
            for h in range(1, N_HOPS):
                vcur = vecT_b if h % 2 == 1 else vecT_a
                vnxt = vecT_a if h % 2 == 1 else vecT_b
                lin_ps = ps.tile([P, 3, BP], F32, tag="accL", space="PSUM")
                lin_mms(vcur, lin_ps)
                svbc_ps = svec_bc(vcur)
                nc.vector.tensor_tensor(out=sc_f[:], in0=msv[:],
                                        in1=svbc_ps[:], op=addop)
                nc.scalar.activation(sc_f[:], sc_f[:], AF.Tanh)
                nc.scalar.activation(sc_f[:], sc_f[:], AF.Exp)
                nc.vector.tensor_tensor(out=e_m[:], in0=sc_f[:],
                                        in1=cmask_ap, op=mult)
                nc.vector.tensor_tensor(out=aw[:], in0=sc_f[:],
                                        in1=cv_ap, op=mult)
                attn_ps = ps.tile([P, 3, BP], F32, tag="accA", space="PSUM")
                cs_ps = ps3.tile([P, 1], F32, tag="psmall", space="PSUM")
                dn_ps = ps3.tile([1, BP], F32, tag="psmall", space="PSUM")
                # The denominator's PE ops interleave with the attention
                # stream; each DVE consumer is emitted right after its
                # producer (cross-engine waits resolve by emission position),
                # so 1/denom is broadcast+staged before the stream ends.
                nc.tensor.matmul(cs_ps[:], lhsT=e_m[:], rhs=ones_ap,
                                 start=True, stop=True)
                nc.vector.tensor_copy(out=cs_sb[:], in_=cs_ps[:])
                for c in range(NCH // 3):
                    attn_mms(attn_ps, aw[:, c:c + 1], c)
                nc.tensor.matmul(dn_ps[:], lhsT=cs_sb[:], rhs=gsel_ap,
                                 start=True, stop=True)
                denom_recip(dn_ps)
                for c in range(NCH // 3, 2 * NCH // 3):
                    attn_mms(attn_ps, aw[:, c:c + 1], c)
                rd_bc = denom_bcast()
                for c in range(2 * NCH // 3, NCH):
                    attn_mms(attn_ps, aw[:, c:c + 1], c)
                assemble(attn_ps, rd_bc, lin_ps, vnxt)
                if DEBUG and h == 1:
                    dbg_v2 = sb.tile([P, 3, BP], F32, tag="dbg_v2_t")
                    nc.vector.memset(dbg_v2[:], 0.0)
                    for k in range(3):
                        nc.vector.tensor_copy(out=dbg_v2[:DK[k], k, :],
                                              in_=vnxt[:DK[k], k, :])
                    nc.sync.dma_start(dbg_v2_d.ap(), dbg_v2[:])

            # ---- output projection ----
            vfin = vecT_b if N_HOPS % 2 == 1 else vecT_a
            lg_ps = ps3.tile([C, BP], F32, tag="psmall", space="PSUM")
            for k in range(3):
                kk = DK[k]
                nc.tensor.matmul(
                    lg_ps[:],
                    lhsT=aux16_sb[:kk, A6_OUTW + k * C:A6_OUTW + (k + 1) * C],
                    rhs=vfin[:kk, k, :], start=(k == 0), stop=(k == 2))
            nc.vector.tensor_scalar_add(lg_sb[:], lg_ps[:],
                                        auxf_sb[0:C, AF_OUTB:AF_OUTB + 1])
            nc.sync.dma_start(out_d.ap(), lg_sb[:])

    nc.compile()
    return nc


def _wrap16(flat):
    """dma_gather index layout: [128, n/16], replicated over 16-row groups."""
    n = flat.shape[0]
    w = flat.reshape(n // 16, 16).T.astype(np.int16)   # [16, n/16]
    return np.ascontiguousarray(np.tile(w, (8, 1)))    # [128, n/16]


def make_core_inputs(context_x, context_len, target_x, target_len, target_loc,
                     emb16, shared):
    """Per-core input dict. context_x etc are the 32-row shards (numpy).

    The embedding table is sharded per core by index compaction: each core
    receives only the (unique) rows its context references, packed as 512B
    rows (300 fp8 values + the precomputed fp16 content score emb@w_mem at
    byte 300), plus int16 local indices in the wrapped dma_gather layout.
    Target rows (160 per core) are materialized host-side in fp16.
    """
    score16 = shared["_score16"]
    flat = np.ascontiguousarray(context_x, dtype=np.int64).reshape(-1)
    uniq, inv = np.unique(flat, return_inverse=True)
    assert uniq.shape[0] <= U_PAD
    emb_loc = np.zeros((U_PAD, EPAD16), np.float16)
    emb_loc[:uniq.shape[0], :D] = emb16[uniq]
    emb_loc[:uniq.shape[0], SCORE_COL] = score16[uniq]
    ctx_idx = _wrap16(inv)

    # host-materialized target rows: row r = j*128 + p -> t = r//32, b = r%32
    tgtr = np.zeros((P, TCOL, TE), np.float16)
    for j in range(TCOL):
        for t0 in range(P // BP):
            t = j * (P // BP) + t0
            if t >= T:
                break
            rows = emb16[target_x[:, t]]                 # [BP, D]
            tgtr[t0 * BP:(t0 + 1) * BP, j, :D] = rows
    tgtr = tgtr.reshape(P, TCOL * TE)

    # host-side location model per (p, c): b = c//4, l = (c%4)*128 + p
    cidx = np.arange(NCH) // CPB
    pos = ((np.arange(NCH)[None, :] % CPB) * P
           + np.arange(P)[:, None]).astype(np.float64)
    loc_b = target_loc[cidx].astype(np.float64)[None, :]
    len_b = context_len[cidx].astype(np.float64)[None, :]
    vloc = 1.0 - np.abs(pos - loc_b) / len_b
    cmask = (pos < len_b).astype(np.float64)

    auxf = np.zeros((P, AF_N), np.float32)
    auxf[:, AF_VLOC:AF_VLOC + NCH] = vloc
    auxf[:, AF_TLEN] = target_len[np.arange(P) % BP]
    auxf[:, AF_T0] = np.arange(P) // BP
    auxf[:, AF_T1] = (P // BP) + np.arange(P) // BP
    auxf[:BP, AF_ID32:AF_ID32 + BP] = np.eye(BP)
    auxf[:, AF_LINB:AF_LINB + 3] = shared["_linb3"]
    auxf[:C, AF_OUTB] = shared["_outb"]
    auxf[0, AF_ATTNB] = shared["_attnb"]
    auxf[:, AF_CMASK:AF_CMASK + NCH] = cmask
    auxf[:, AF_CV:AF_CV + NCH] = cmask * vloc

    d = dict(aux16_h=shared["aux16_h"], lin_w_h=shared["lin_w_h"])
    d.update(emb_loc=emb_loc, ctx_idx16=ctx_idx, tgtr_h=tgtr, auxf_h=auxf)
    return d


def make_shared_inputs(emb, attn_w, attn_b, lin_w, lin_b, out_w, out_b):
    lin_w_pad = np.zeros((384, 384), np.float16)
    lin_w_pad[:D, :D] = lin_w.astype(np.float16)
    lin_w_h = np.ascontiguousarray(
        lin_w_pad.reshape(3, P, 384).transpose(1, 0, 2).reshape(P, 3 * 384))

    aux16 = np.zeros((P, A6_N), np.float16)
    aux16[:, A6_GSEL:A6_GSEL + BP] = (
        np.arange(P)[:, None] // CPB == np.arange(BP)[None, :])
    aux16[:, A6_SSEL:A6_SSEL + BP] = (
        np.arange(P)[:, None] % BP == np.arange(BP)[None, :])
    w_vec_pad = np.zeros((384,), np.float16)
    w_vec_pad[:D] = attn_w[D:, 0].astype(np.float16)
    aux16[:, A6_WVEC:A6_WVEC + 3] = w_vec_pad.reshape(3, P).T
    aux16[:, A6_ONES] = 1.0
    aux16[0, A6_ONESR:A6_ONESR + P] = 1.0
    out_w_pad = np.zeros((384, C), np.float16)
    out_w_pad[:D] = out_w.astype(np.float16)
    aux16[:, A6_OUTW:A6_OUTW + 3 * C] = (
        out_w_pad.reshape(3, P, C).transpose(1, 0, 2).reshape(P, 3 * C))

    lin_b_pad = np.zeros((384,), np.float32)
    lin_b_pad[:D] = lin_b
    score16 = (np.asarray(emb, np.float64)
               @ np.asarray(attn_w[:D, 0], np.float64)).astype(np.float16)
    return dict(
        lin_w_h=lin_w_h,
        aux16_h=aux16,
        _linb3=np.ascontiguousarray(lin_b_pad.reshape(3, P).T),
        _outb=out_b.astype(np.float32),
        _attnb=np.float32(attn_b[0]),
        _score16=score16,
    )


_module_cache = {}


def get_module():
    if "nc" not in _module_cache:
        _module_cache["nc"] = build_module()
    return _module_cache["nc"]


def kernel(**inputs):
    emb16 = np.ascontiguousarray(inputs["emb"].astype(np.float16))
    shared = make_shared_inputs(
        np.asarray(inputs["emb"]), np.asarray(inputs["attn_w"]),
        np.asarray(inputs["attn_b"]), np.asarray(inputs["lin_w"]),
        np.asarray(inputs["lin_b"]), np.asarray(inputs["out_w"]),
        np.asarray(inputs["out_b"]))
    in_maps = []
    for k in range(N_CORES):
        s = slice(k * BP, (k + 1) * BP)
        in_maps.append(make_core_inputs(
            np.asarray(inputs["context_x"])[s],
            np.asarray(inputs["context_len"])[s],
            np.asarray(inputs["target_x"])[s],
            np.asarray(inputs["target_len"])[s],
            np.asarray(inputs["target_loc"])[s],
            emb16, shared))
    nc = get_module()
    res = bass_utils.run_bass_kernel_spmd(nc, in_maps,
                                          core_ids=list(range(N_CORES)))
    out = np.concatenate([res.results[k]["logits_t"].T
                          for k in range(N_CORES)], axis=0)
    return out.astype(np.float32)


# revision 55
# speedup vs baseline: 1.2968x; 1.2552x over previous
"""MemNet (scatter_memory) Trainium2 kernel, v2.

Model (per batch row b):
  memory   = emb[context_x[b]]                    # [L, D] gather
  v_aspect = masked-mean(emb[target_x[b]])        # [D]
  v_loc    = 1 - |pos - target_loc[b]| / context_len[b]
  3 hops of: scores = tanh((memory*v_loc) @ w_mem + vec@w_vec + b)
             alpha  = masked softmax;  vec = alpha @ (memory*v_loc) + vec@lin_w+lin_b
  logits   = vec @ out_w + out_b

Sharding: data-parallel over batch, 32 rows per core on 8 cores; the fp16
embedding table is index-compacted per core and fetched by indirect DMA
gather.

Key structure (vs v1):
- The content score emb@w_mem is a pure weight transform; it is appended
  as column 300 of the gathered rows (inside the 768B alignment padding),
  so scores arrive with the gather for free.
- Attention contraction runs transposed on the PE: per (chunk, d-slice),
  stationary = mem[128, DK], moving = the chunk's alpha column [128, 1],
  accumulating over each batch row's 4 chunks directly into vec^T layout
  [DK, b] in PSUM. Cost ~ 1 cycle per matmul (output free size 1).
- v_loc/cmask are host-side index-derived tensors; hop 1 (whose alpha
  depends only on v_aspect) is pipelined per gather group so its attention
  runs under the gather phase; the softmax denominator accumulates per
  group in an open PSUM matmul group.
"""

import numpy as np

import concourse.bass as bass
import concourse.bacc as bacc
import concourse.mybir as mybir
import concourse.tile as tile
from concourse import bass_utils

N_CORES = 8
B, L, T, V, D, C = 256, 512, 5, 50000, 300, 3
N_HOPS = 3
BP = B // N_CORES          # 32 batch rows per core
P = 128                    # partitions
NCH = (BP * L) // P        # 128 chunk columns; chunk c: b=c//4, l=(c%4)*128+p
CPB = L // P               # 4 chunks per batch row
NGRP = 16                  # gather groups (<=1024 idxs per dma_gather)
GW = NCH // NGRP           # chunk columns per gather group (8)
DK = [128, 128, 44]        # D split across PSUM partition chunks
DOF = [0, 128, 256]
TCOL = (BP * T + P - 1) // P  # 2 columns of host-provided target rows
EPAD16 = 384               # row length in f16 units (768B rows: 300 fp16
                           # values + fp16 content score, 256B-aligned)
SCORE_COL = 300            # f16 column of the packed content score
TE = 304                   # target row pad (f16 units)
U_PAD = 16768              # fixed local-table rows (>= 16384)

F16 = mybir.dt.float16
I16 = mybir.dt.int16
F32 = mybir.dt.float32
F8 = mybir.dt.float8e4

# auxf (f32) column layout
AF_VLOC = 0            # [0:128)   vloc
AF_TLEN = 128          # target_len per partition (p % 32)
AF_T0 = 129            # target t-index for j=0 (p // 32)
AF_T1 = 130            # target t-index for j=1 (4 + p // 32)
AF_ID32 = 131          # [131:163) id32 (rows 0:32)
AF_LINB = 163          # [163:166) lin_b d-chunks
AF_OUTB = 166          # out_b (rows 0:3)
AF_ATTNB = 167         # attn_b (row 0)
AF_CMASK = 168         # [168:296) cmask
AF_CV = 296            # [296:424) cmask * vloc
AF_TLENP = 424         # target_len per slot (rows 0:32)
AF_SSEL = 425          # [425:457) ssel: target row p -> slot column
AF_N = 457

# aux16 (f16) column layout
A6_GSEL = 0            # [0:32)   gsel: chunk c (partition) -> b
A6_SSEL = 32           # [32:64)  ssel: target row p -> b = p % 32
A6_WVEC = 64           # [64:67)  w_vec d-chunks
A6_ONES = 67           # ones column
A6_ONESR = 68          # [68:196) ones row (partition 0)
A6_OUTW = 196          # [196:205) out_w  [d-part, 3 k, C]
A6_N = 205


def _ap2d(tile_ap, col_off, stride, n):
    """2D AP over a 3D tile: partition dim + one strided free dim."""
    return bass.AP(tile_ap.tensor, tile_ap.offset + col_off,
                   [list(tile_ap.ap[0]), [stride, n]])


def _row_rep4(ap2):
    """[1, BP] row AP -> [1, CPB, BP] with the outer dim broadcast (step 0),
    so column c = 32*r + j reads value j."""
    return bass.AP(ap2.tensor, ap2.offset,
                   [list(ap2.ap[0]), [0, CPB], [1, BP]])


DEBUG = False


def build_module(m=(BP, BP, BP)):
    """m = (m1, m2, m3): valid slot count per l-band r=1..3 (band 0 is always
    full). Chunk column c = 32*r + j holds l in [128r, 128r+128) of the
    batch in slot j (host sorts batches by descending context_len, so valid
    chunks are a prefix of each band); only valid chunks are gathered.
    """
    m_band = [BP, m[0], m[1], m[2]]
    # per-gather-group valid widths (group g = columns [8g, 8g+8))
    gw = [max(0, min(GW, m_band[(8 * g) // BP] - (8 * g) % BP))
          for g in range(NGRP)]
    vcols = [8 * g + cc for g in range(NGRP) for cc in range(gw[g])]

    nc = bacc.Bacc("TRN2", target_bir_lowering=False, debug=False,
                   num_devices=N_CORES)

    emb_d = nc.dram_tensor("emb_loc", [U_PAD, EPAD16], F16,
                           kind="ExternalInput")
    ctx_idx_d = nc.dram_tensor("ctx_idx16", [P, NCH * P // 16], I16,
                               kind="ExternalInput")
    tgtr_d = nc.dram_tensor("tgtr_h", [P, TCOL * TE], F16,
                            kind="ExternalInput")
    auxf_d = nc.dram_tensor("auxf_h", [P, AF_N], F32, kind="ExternalInput")
    aux16_d = nc.dram_tensor("aux16_h", [P, A6_N], F16, kind="ExternalInput")
    linw_d = nc.dram_tensor("lin_w_h", [P, 3 * 384], F16, kind="ExternalInput")

    out_d = nc.dram_tensor("logits_t", [C, BP], F32, kind="ExternalOutput")
    if DEBUG:
        dbg_msv_d = nc.dram_tensor("dbg_msv", [P, NCH], F32,
                                   kind="ExternalOutput")
        dbg_em_d = nc.dram_tensor("dbg_em", [P, NCH], F32,
                                  kind="ExternalOutput")
        dbg_va_d = nc.dram_tensor("dbg_va", [BP, D], F32,
                                  kind="ExternalOutput")
        dbg_v1_d = nc.dram_tensor("dbg_v1", [P, 3 * BP], F32,
                                  kind="ExternalOutput")
        dbg_v2_d = nc.dram_tensor("dbg_v2", [P, 3 * BP], F32,
                                  kind="ExternalOutput")

    mult = mybir.AluOpType.mult
    addop = mybir.AluOpType.add
    is_lt = mybir.AluOpType.is_lt
    AF = mybir.ActivationFunctionType

    with tile.TileContext(nc) as tc:
        with (
            tc.tile_pool(name="sb", bufs=1) as sb,
            tc.tile_pool(name="sc", bufs=4) as scr,
            tc.tile_pool(name="ps", bufs=1, space="PSUM") as ps,
            tc.tile_pool(name="ps3", bufs=2, space="PSUM") as ps3,
        ):
            # ---- persistent SBUF tiles ----
            idx_sb = sb.tile([P, NCH * P // 16], I16, tag="idx")
            auxf_sb = sb.tile([P, AF_N], F32, tag="auxf")
            aux16_sb = sb.tile([P, A6_N], F16, tag="aux16")
            linw_sb = sb.tile([P, 3, 384], F16, tag="linw")
            mem_sb = [sb.tile([P, GW, EPAD16], F16, tag=f"mem{g}",
                              name=f"mem{g}") for g in range(NGRP)]
            tgtr_sb = sb.tile([P, TCOL, TE], F16, tag="tgtr")

            tmask = sb.tile([P, TCOL], F32, tag="tmask")
            a0 = sb.tile([P, BP, TCOL], F16, tag="a0")
            tlenr = sb.tile([BP, 1], F32, tag="tlenr")
            va_sb = sb.tile([BP, D], F32, tag="va")
            vecT_a = sb.tile([P, 3, BP], F16, tag="vecTa", name="vecT_a")
            vecT_b = sb.tile([P, 3, BP], F16, tag="vecTb", name="vecT_b")
            msv = sb.tile([P, NCH], F32, tag="msv")
            sc_f = sb.tile([P, NCH], F32, tag="scf")
            e_m = sb.tile([P, NCH], F16, tag="em")
            aw = sb.tile([P, NCH], F16, tag="aw")
            aw1 = [sb.tile([P, GW], F16, tag=f"aw1_{g}", name=f"aw1_{g}")
                   for g in range(NGRP)]
            svec_sb = sb.tile([1, BP], F16, tag="svec")
            cs_sb = sb.tile([P, 1], F16, tag="cs")
            rdr_sb = sb.tile([1, BP], F16, tag="rdr")
            rd_sb = sb.tile([P, BP], F32, tag="rdbc_s")
            asm3 = sb.tile([P, 3, BP], F32, tag="asm")
            lg_sb = sb.tile([C, BP], F32, tag="lg")

            vloc_ap = auxf_sb[:, AF_VLOC:AF_VLOC + NCH]
            cmask_ap = auxf_sb[:, AF_CMASK:AF_CMASK + NCH]
            cv_ap = auxf_sb[:, AF_CV:AF_CV + NCH]
            gsel_ap = aux16_sb[:, A6_GSEL:A6_GSEL + BP]
            ssel_ap = auxf_sb[:, AF_SSEL:AF_SSEL + BP]
            ones_ap = aux16_sb[:, A6_ONES:A6_ONES + 1]
            onesr_ap = aux16_sb[0:1, A6_ONESR:A6_ONESR + P]
            id32_ap = auxf_sb[0:BP, AF_ID32:AF_ID32 + BP]

            # ---- input DMAs (context indices first so gathers start early) ----
            nc.sync.dma_start(idx_sb[:], ctx_idx_d.ap())
            nc.sync.dma_start(tgtr_sb[:], tgtr_d.ap())
            nc.sync.dma_start(auxf_sb[:], auxf_d.ap())
            nc.sync.dma_start(aux16_sb[:], aux16_d.ap())
            nc.sync.dma_start(linw_sb[:], linw_d.ap())

            # uninvolved score slots must stay finite: zero msv/e_m once so
            # chunks never gathered (invalid/pad) read as 0 through the
            # softmax (cmask/cv are 0 there host-side)
            nc.vector.memset(msv[:], 0.0)
            nc.vector.memset(e_m[:], 0.0)

            # ---- gathers (768B rows: fp16 values + fp16 score), only the
            # valid prefix of each group ----
            NIC = GW * P // 16  # idx tile columns per full group
            for g in range(NGRP):
                if gw[g] == 0:
                    continue
                nig = gw[g] * P
                nc.gpsimd.dma_gather(
                    out_ap=mem_sb[g][:, 0:gw[g], :], in_ap=emb_d.ap(),
                    idxs_ap=idx_sb[:, g * NIC:g * NIC + nig // 16],
                    num_idxs=nig, num_idxs_reg=nig, elem_size=EPAD16)

            # ---- v_aspect -> vecT_a ----
            nc.vector.tensor_tensor(
                out=tmask[:], in0=auxf_sb[:, AF_T0:AF_T0 + TCOL],
                in1=auxf_sb[:, AF_TLEN:AF_TLEN + 1].to_broadcast([P, TCOL]),
                op=is_lt)
            va_ps = ps.tile([BP, D], F32, tag="acc300", space="PSUM")
            for j in range(TCOL):
                nc.vector.tensor_scalar_mul(a0[:, :, j], ssel_ap,
                                            tmask[:, j:j + 1])
                nc.tensor.matmul(va_ps[:], lhsT=a0[:, :, j],
                                 rhs=tgtr_sb[:, j, 0:D],
                                 start=(j == 0), stop=(j == TCOL - 1))
            nc.vector.reciprocal(tlenr[:], auxf_sb[0:BP, AF_TLENP:AF_TLENP + 1])
            nc.vector.tensor_scalar_mul(va_sb[:], va_ps[:], tlenr[:])
            for k in range(3):
                kk = DK[k]
                t_ps = ps3.tile([P, BP], F32, tag="psmall", space="PSUM")
                nc.tensor.transpose(t_ps[:kk, :], va_sb[:, DOF[k]:DOF[k] + kk],
                                    id32_ap)
                nc.vector.tensor_copy(out=vecT_a[:kk, k, :], in_=t_ps[:kk, :])

            # PSUM accumulation semantics: start=True lazily zeroes the whole
            # 2KB bank (each byte is overwritten by its first write after the
            # start). So each PSUM tile gets exactly ONE start (first matmul)
            # and ONE stop (last matmul); disjoint sub-regions accumulate
            # independently in between.
            def lin_mms(vcur, lin_ps):
                for k in (0, 2, 1):  # 128-partition slice first and last
                    kk = DK[k]
                    for kx in range(3):
                        kkx = DK[kx]
                        nc.tensor.matmul(
                            lin_ps[:kk, k, :],
                            lhsT=linw_sb[:kkx, kx, DOF[k]:DOF[k] + kk],
                            rhs=vcur[:kkx, kx, :],
                            start=(k == 0 and kx == 0),
                            stop=(k == 1 and kx == 2))

            def svec_bc(vcur):
                """svec = vec @ w_vec + attn_b, broadcast to [P, NCH]."""
                svec_ps = ps3.tile([1, BP], F32, tag="psmall", space="PSUM")
                for k in range(3):
                    kk = DK[k]
                    nc.tensor.matmul(svec_ps[:],
                                     lhsT=aux16_sb[:kk, A6_WVEC + k:A6_WVEC + k + 1],
                                     rhs=vcur[:kk, k, :],
                                     start=(k == 0), stop=(k == 2))
                nc.vector.tensor_scalar_add(svec_sb[:], svec_ps[:],
                                            auxf_sb[0:1, AF_ATTNB:AF_ATTNB + 1])
                svbc_ps = ps.tile([P, NCH], F32, tag="svbc", space="PSUM")
                nc.tensor.matmul(svbc_ps[:], lhsT=onesr_ap,
                                 rhs=_row_rep4(svec_sb[:]),
                                 start=True, stop=True)
                return svbc_ps

            def attn_mms(attn_ps, alpha_col, c):
                """3 accumulating matmuls: attn^T[:, b] += mem_c^T-slices @ alpha.

                The whole [P, 3, BP] tile is one PSUM group per hop: start on
                the very first matmul, stop on the very last (see note above).
                """
                g, cc = divmod(c, GW)
                b = c % BP
                # k order (0, 2, 1): the group's first AND last matmul must
                # span all 128 partitions (start/stop flag the bank on the
                # instruction's own partition range only)
                for k in (0, 2, 1):
                    kk = DK[k]
                    nc.tensor.matmul(
                        attn_ps[:kk, k, b:b + 1],
                        lhsT=mem_sb[g][:, cc, DOF[k]:DOF[k] + DK[k]],
                        rhs=alpha_col,
                        start=(c == vcols[0] and k == 0),
                        stop=(c == vcols[-1] and k == 1))

            def denom_recip(dn_ps):
                """reciprocal of the softmax denominator row."""
                with nc.allow_low_precision(reason="fp16 1/denom, rel 5e-4"):
                    nc.vector.reciprocal(rdr_sb[:], dn_ps[:])

            def denom_bcast():
                """broadcast 1/denom across partitions, staged to SBUF (HW: a
                DVE op may read at most one PSUM operand)."""
                rd_ps = ps3.tile([P, BP], F32, tag="psmall", space="PSUM")
                nc.tensor.matmul(rd_ps[:], lhsT=onesr_ap, rhs=rdr_sb[:],
                                 start=True, stop=True)
                nc.vector.tensor_copy(out=rd_sb[:], in_=rd_ps[:])
                return rd_sb

            def assemble(attn_ps, rd_bc, lin_ps, vnxt):
                # k=1 first: its PSUM reads depend on both groups' stop
                # matmuls (the k=1 sections), so the later k=2/k=0 reads on
                # the in-order DVE are also past them. Phase A ops are
                # mutually independent; each phase-B op depends on a phase-A
                # op 3 slots back, so the RAW sem is already satisfied.
                for k in (1, 2, 0):
                    kk = DK[k]
                    nc.vector.tensor_tensor(out=asm3[:kk, k, :],
                                            in0=attn_ps[:kk, k, :],
                                            in1=rd_bc[:kk, :], op=mult)
                for k in (1, 2, 0):
                    kk = DK[k]
                    nc.vector.scalar_tensor_tensor(
                        out=vnxt[:kk, k, :], in0=lin_ps[:kk, k, :],
                        scalar=auxf_sb[:kk, AF_LINB + k:AF_LINB + k + 1],
                        in1=asm3[:kk, k, :], op0=addop, op1=addop)

            # ======== hop 1 (pipelined per gather group) ========
            lin_ps = ps.tile([P, 3, BP], F32, tag="accL", space="PSUM")
            lin_mms(vecT_a, lin_ps)
            svbc_ps = svec_bc(vecT_a)
            attn_ps = ps.tile([P, 3, BP], F32, tag="accA", space="PSUM")
            for g in range(NGRP):
                w = gw[g]
                if w == 0:
                    continue
                gs = g * GW
                score_ap = _ap2d(mem_sb[g][:], SCORE_COL, EPAD16, w)
                nc.vector.tensor_tensor(out=msv[:, gs:gs + w], in0=score_ap,
                                        in1=vloc_ap[:, gs:gs + w], op=mult)
                st = scr.tile([P, GW], F32, tag="st", bufs=4)
                nc.vector.tensor_tensor(out=st[:, 0:w], in0=msv[:, gs:gs + w],
                                        in1=svbc_ps[:, gs:gs + w], op=addop)
                nc.scalar.activation(st[:, 0:w], st[:, 0:w], AF.Tanh)
                nc.scalar.activation(st[:, 0:w], st[:, 0:w], AF.Exp)
                nc.vector.tensor_tensor(out=e_m[:, gs:gs + w], in0=st[:, 0:w],
                                        in1=cmask_ap[:, gs:gs + w], op=mult)
                nc.vector.tensor_tensor(out=aw1[g][:, 0:w], in0=st[:, 0:w],
                                        in1=cv_ap[:, gs:gs + w], op=mult)
                for cc in range(w):
                    attn_mms(attn_ps, aw1[g][:, cc:cc + 1], gs + cc)
            cs_ps = ps3.tile([P, 1], F32, tag="psmall", space="PSUM")
            nc.tensor.matmul(cs_ps[:], lhsT=e_m[:], rhs=ones_ap,
                             start=True, stop=True)
            nc.vector.tensor_copy(out=cs_sb[:], in_=cs_ps[:])
            dn_ps = ps3.tile([1, BP], F32, tag="psmall", space="PSUM")
            nc.tensor.matmul(dn_ps[:], lhsT=cs_sb[:], rhs=gsel_ap,
                             start=True, stop=True)
            denom_recip(dn_ps)
            rd_bc = denom_bcast()
            assemble(attn_ps, rd_bc, lin_ps, vecT_b)

            if DEBUG:
                dbg_va = sb.tile([BP, D], F32, tag="dbg_va_t")
                nc.vector.tensor_copy(out=dbg_va[:], in_=va_sb[:])
                nc.sync.dma_start(dbg_va_d.ap(), dbg_va[:])
                dbg_msv = sb.tile([P, NCH], F32, tag="dbg_msv_t")
                nc.vector.tensor_copy(out=dbg_msv[:], in_=msv[:])
                nc.sync.dma_start(dbg_msv_d.ap(), dbg_msv[:])
                dbg_em = sb.tile([P, NCH], F32, tag="dbg_em_t")
                nc.vector.tensor_copy(out=dbg_em[:], in_=e_m[:])
                nc.sync.dma_start(dbg_em_d.ap(), dbg_em[:])
                dbg_v1 = sb.tile([P, 3, BP], F32, tag="dbg_v1_t")
                nc.vector.memset(dbg_v1[:], 0.0)
                for k in range(3):
                    nc.vector.tensor_copy(out=dbg_v1[:DK[k], k, :],
                                          in_=vecT_b[:DK[k], k, :])
                nc.sync.dma_start(dbg_v1_d.ap(), dbg_v1[:])

            # ======== hops 2..N ========
            for h in range(1, N_HOPS):
                vcur = vecT_b if h % 2 == 1 else vecT_a
                vnxt = vecT_a if h % 2 == 1 else vecT_b
                lin_ps = ps.tile([P, 3, BP], F32, tag="accL", space="PSUM")
                lin_mms(vcur, lin_ps)
                svbc_ps = svec_bc(vcur)
                nc.vector.tensor_tensor(out=sc_f[:], in0=msv[:],
                                        in1=svbc_ps[:], op=addop)
                nc.scalar.activation(sc_f[:], sc_f[:], AF.Tanh)
                nc.scalar.activation(sc_f[:], sc_f[:], AF.Exp)
                nc.vector.tensor_tensor(out=e_m[:], in0=sc_f[:],
                                        in1=cmask_ap, op=mult)
                nc.vector.tensor_tensor(out=aw[:], in0=sc_f[:],
                                        in1=cv_ap, op=mult)
                attn_ps = ps.tile([P, 3, BP], F32, tag="accA", space="PSUM")
                cs_ps = ps3.tile([P, 1], F32, tag="psmall", space="PSUM")
                dn_ps = ps3.tile([1, BP], F32, tag="psmall", space="PSUM")
                # The denominator's PE ops interleave with the attention
                # stream; each DVE consumer is emitted right after its
                # producer (cross-engine waits resolve by emission position),
                # so 1/denom is broadcast+staged before the stream ends.
                nc.tensor.matmul(cs_ps[:], lhsT=e_m[:], rhs=ones_ap,
                                 start=True, stop=True)
                nc.vector.tensor_copy(out=cs_sb[:], in_=cs_ps[:])
                nv = len(vcols)
                for c in vcols[:nv // 3]:
                    attn_mms(attn_ps, aw[:, c:c + 1], c)
                nc.tensor.matmul(dn_ps[:], lhsT=cs_sb[:], rhs=gsel_ap,
                                 start=True, stop=True)
                denom_recip(dn_ps)
                for c in vcols[nv // 3:2 * nv // 3]:
                    attn_mms(attn_ps, aw[:, c:c + 1], c)
                rd_bc = denom_bcast()
                for c in vcols[2 * nv // 3:]:
                    attn_mms(attn_ps, aw[:, c:c + 1], c)
                assemble(attn_ps, rd_bc, lin_ps, vnxt)
                if DEBUG and h == 1:
                    dbg_v2 = sb.tile([P, 3, BP], F32, tag="dbg_v2_t")
                    nc.vector.memset(dbg_v2[:], 0.0)
                    for k in range(3):
                        nc.vector.tensor_copy(out=dbg_v2[:DK[k], k, :],
                                              in_=vnxt[:DK[k], k, :])
                    nc.sync.dma_start(dbg_v2_d.ap(), dbg_v2[:])

            # ---- output projection ----
            vfin = vecT_b if N_HOPS % 2 == 1 else vecT_a
            lg_ps = ps3.tile([C, BP], F32, tag="psmall", space="PSUM")
            for k in range(3):
                kk = DK[k]
                nc.tensor.matmul(
                    lg_ps[:],
                    lhsT=aux16_sb[:kk, A6_OUTW + k * C:A6_OUTW + (k + 1) * C],
                    rhs=vfin[:kk, k, :], start=(k == 0), stop=(k == 2))
            nc.vector.tensor_scalar_add(lg_sb[:], lg_ps[:],
                                        auxf_sb[0:C, AF_OUTB:AF_OUTB + 1])
            nc.sync.dma_start(out_d.ap(), lg_sb[:])

    nc.compile()
    return nc


def _slot_order(context_len):
    """Slot permutation: batches sorted by descending context_len."""
    return np.argsort(-np.asarray(context_len), kind="stable")


def _band_counts(context_len):
    """(m1, m2, m3): #batches with len > 128r for r = 1, 2, 3."""
    cl = np.asarray(context_len)
    return tuple(int((cl > 128 * r).sum()) for r in (1, 2, 3))


def _wrap16(flat):
    """dma_gather index layout: [128, n/16], replicated over 16-row groups."""
    n = flat.shape[0]
    w = flat.reshape(n // 16, 16).T.astype(np.int16)   # [16, n/16]
    return np.ascontiguousarray(np.tile(w, (8, 1)))    # [128, n/16]


def make_core_inputs(context_x, context_len, target_x, target_len, target_loc,
                     emb16, shared):
    """Per-core input dict. context_x etc are the 32-row shards (numpy).

    The embedding table is sharded per core by index compaction: each core
    receives only the (unique) rows its context references, packed as 512B
    rows (300 fp8 values + the precomputed fp16 content score emb@w_mem at
    byte 300), plus int16 local indices in the wrapped dma_gather layout.
    Target rows (160 per core) are materialized host-side in fp16.
    """
    score16 = shared["_score16"]
    order = _slot_order(context_len)
    # flat gather stream in (c, p) order: chunk c = 32r + j holds
    # l in [128r, 128r+128) of batch order[j]
    cj = np.arange(NCH) % BP
    cr = np.arange(NCH) // BP
    bmap = order[cj]                                       # [NCH]
    flat = np.zeros((NCH, P), np.int64)
    for c in range(NCH):
        flat[c] = context_x[bmap[c], cr[c] * P:(cr[c] + 1) * P]
    flat = flat.reshape(-1)
    uniq, inv = np.unique(flat, return_inverse=True)
    assert uniq.shape[0] <= U_PAD
    emb_loc = np.zeros((U_PAD, EPAD16), np.float16)
    emb_loc[:uniq.shape[0], :D] = emb16[uniq]
    emb_loc[:uniq.shape[0], SCORE_COL] = score16[uniq]
    ctx_idx = _wrap16(inv)

    # host-materialized target rows: row r = j*128 + p -> t = r//32, b = r%32
    tgtr = np.zeros((P, TCOL, TE), np.float16)
    for j in range(TCOL):
        for t0 in range(P // BP):
            t = j * (P // BP) + t0
            if t >= T:
                break
            rows = emb16[target_x[:, t]]                 # [BP, D]
            tgtr[t0 * BP:(t0 + 1) * BP, j, :D] = rows
    tgtr = tgtr.reshape(P, TCOL * TE)

    # host-side location model per (p, c): c = 32r + j -> b = order[j],
    # l = 128r + p
    pos = (cr[None, :] * P + np.arange(P)[:, None]).astype(np.float64)
    loc_b = target_loc[bmap].astype(np.float64)[None, :]
    len_b = context_len[bmap].astype(np.float64)[None, :]
    vloc = 1.0 - np.abs(pos - loc_b) / len_b
    cmask = (pos < len_b).astype(np.float64)

    auxf = np.zeros((P, AF_N), np.float32)
    auxf[:, AF_VLOC:AF_VLOC + NCH] = vloc
    auxf[:, AF_TLEN] = target_len[np.arange(P) % BP]
    auxf[:, AF_T0] = np.arange(P) // BP
    auxf[:, AF_T1] = (P // BP) + np.arange(P) // BP
    auxf[:BP, AF_ID32:AF_ID32 + BP] = np.eye(BP)
    auxf[:, AF_LINB:AF_LINB + 3] = shared["_linb3"]
    auxf[:C, AF_OUTB] = shared["_outb"]
    auxf[0, AF_ATTNB] = shared["_attnb"]
    auxf[:, AF_CMASK:AF_CMASK + NCH] = cmask
    auxf[:, AF_CV:AF_CV + NCH] = cmask * vloc
    auxf[:BP, AF_TLENP] = target_len[order]
    auxf[:, AF_SSEL:AF_SSEL + BP] = (
        np.arange(P)[:, None] % BP == order[None, :])

    d = dict(aux16_h=shared["aux16_h"], lin_w_h=shared["lin_w_h"])
    d.update(emb_loc=emb_loc, ctx_idx16=ctx_idx, tgtr_h=tgtr, auxf_h=auxf)
    return d


def make_shared_inputs(emb, attn_w, attn_b, lin_w, lin_b, out_w, out_b):
    lin_w_pad = np.zeros((384, 384), np.float16)
    lin_w_pad[:D, :D] = lin_w.astype(np.float16)
    lin_w_h = np.ascontiguousarray(
        lin_w_pad.reshape(3, P, 384).transpose(1, 0, 2).reshape(P, 3 * 384))

    aux16 = np.zeros((P, A6_N), np.float16)
    # gsel: chunk (partition) c -> slot column c % 32
    aux16[:, A6_GSEL:A6_GSEL + BP] = (
        np.arange(P)[:, None] % BP == np.arange(BP)[None, :])
    w_vec_pad = np.zeros((384,), np.float16)
    w_vec_pad[:D] = attn_w[D:, 0].astype(np.float16)
    aux16[:, A6_WVEC:A6_WVEC + 3] = w_vec_pad.reshape(3, P).T
    aux16[:, A6_ONES] = 1.0
    aux16[0, A6_ONESR:A6_ONESR + P] = 1.0
    out_w_pad = np.zeros((384, C), np.float16)
    out_w_pad[:D] = out_w.astype(np.float16)
    aux16[:, A6_OUTW:A6_OUTW + 3 * C] = (
        out_w_pad.reshape(3, P, C).transpose(1, 0, 2).reshape(P, 3 * C))

    lin_b_pad = np.zeros((384,), np.float32)
    lin_b_pad[:D] = lin_b
    score16 = (np.asarray(emb, np.float64)
               @ np.asarray(attn_w[:D, 0], np.float64)).astype(np.float16)
    return dict(
        lin_w_h=lin_w_h,
        aux16_h=aux16,
        _linb3=np.ascontiguousarray(lin_b_pad.reshape(3, P).T),
        _outb=out_b.astype(np.float32),
        _attnb=np.float32(attn_b[0]),
        _score16=score16,
    )


_module_cache = {}


def get_module(m=None):
    if m is None:
        # most-recently built module (test.py convenience)
        return next(reversed(_module_cache.values()))
    if m not in _module_cache:
        _module_cache[m] = build_module(m)
    return _module_cache[m]


def kernel(**inputs):
    emb16 = np.ascontiguousarray(inputs["emb"].astype(np.float16))
    shared = make_shared_inputs(
        np.asarray(inputs["emb"]), np.asarray(inputs["attn_w"]),
        np.asarray(inputs["attn_b"]), np.asarray(inputs["lin_w"]),
        np.asarray(inputs["lin_b"]), np.asarray(inputs["out_w"]),
        np.asarray(inputs["out_b"]))
    context_len = np.asarray(inputs["context_len"])
    in_maps = []
    orders = []
    mm = (0, 0, 0)
    for k in range(N_CORES):
        s = slice(k * BP, (k + 1) * BP)
        in_maps.append(make_core_inputs(
            np.asarray(inputs["context_x"])[s],
            context_len[s],
            np.asarray(inputs["target_x"])[s],
            np.asarray(inputs["target_len"])[s],
            np.asarray(inputs["target_loc"])[s],
            emb16, shared))
        orders.append(_slot_order(context_len[s]))
        mm = tuple(max(a, b) for a, b in
                   zip(mm, _band_counts(context_len[s])))
    nc = get_module(mm)
    res = bass_utils.run_bass_kernel_spmd(nc, in_maps,
                                          core_ids=list(range(N_CORES)))
    out = np.empty((B, C), np.float32)
    for k in range(N_CORES):
        blk = res.results[k]["logits_t"].T.astype(np.float32)  # [slot, C]
        out[k * BP + orders[k]] = blk
    return out
